# revision 1
# baseline (speedup 1.0000x reference)
"""AlphaModel (relation-gated message passing) Trainium2 kernel.

Strategy (pure data parallel, per sharding hint):
  - Shard the 8M edges across 8 NeuronCores (1M each, zero-padded to a tile
    multiple).
  - Host precomputes g = concat(M.reshape(64,9), beta)[rels]  (tiny-table row
    gather, 12 f32 per edge) and streams it; there is no per-element LUT
    primitive on TRN2 that beats streaming (PE is 1 col/cycle, Pool gather
    ucode ~14 cyc/edge, indirect DMA ~0.34ns/descriptor).
  - Device does everything else: 3x3 matvec, sparsemax (via the simplex
    projection identity tau = max(mx-1, (sm-mn-1)/2, (sm-1)/3)), entropy,
    cosine similarity, scaling - in fp32 planar layout with custom fused DVE
    ops plus ACT for Ln/Sqrt/Square.

Output: alpha [8M, 3] float32.
"""

import sys

if "/opt/trn_rl_repo" not in sys.path:
    sys.path.insert(0, "/opt/trn_rl_repo")

import numpy as np

import concourse.bacc as bacc
import concourse.mybir as mybir
from concourse.bass_utils import run_bass_kernel_spmd
from concourse.tile import TileContext

N_CORES = 8
PDIM = 128

AF = mybir.ActivationFunctionType
OP = mybir.AluOpType
F32 = mybir.dt.float32

# --------------------------------------------------------------------------
# Custom fused DVE ops (registered once per process; compiled into the NEFF's
# per-kernel DVE table - documented extension point, no firmware change).
# --------------------------------------------------------------------------
_OPS_CACHE: dict = {}


def _custom_ops():
    if _OPS_CACHE:
        return _OPS_CACHE
    from concourse import dve_ops
    from concourse.dve_ops import DveOp, OPS, _SUB_OPCODE_FOR_NAME
    from concourse.dve_spec import (
        C0,
        C1,
        One,
        Spec,
        Src0,
        Src1,
        _has_src1,
        lower,
        maxx,
        relu,
    )
    from concourse.dve_uop import DveOpSpec

    existing = {op.name: op for op in OPS}

    def mk(key, name, body):
        if name in existing:
            _OPS_CACHE[key] = existing[name]
            return
        if name not in _SUB_OPCODE_FOR_NAME:
            row = max(_SUB_OPCODE_FOR_NAME.values()) + 1
            assert row < 0x20, "custom DVE opcode rows exhausted"
            _SUB_OPCODE_FOR_NAME[name] = row
        spec = Spec(body=body)
        shas = {}
        for ver in ("v3", "v4"):
            uops = lower(spec, ver=ver)
            s = DveOpSpec(
                name=name,
                opcode=_SUB_OPCODE_FOR_NAME[name],
                uops=uops,
                rd1_en=_has_src1(spec),
            )
            shas[ver] = s.sha(ver)
        op = DveOp(name, spec, subdim=False, uops_sha=shas)
        OPS.append(op)
        dve_ops.CUSTOM_DVE_SPECS[name] = spec
        _OPS_CACHE[key] = op

    # tau candidates: max((sm - mn - 1)*0.5, (sm - 1)/3)
    mk("tau_a", "ANT_TAU_A", maxx((Src0 - Src1 - One) * C0, (Src0 - One) * C1))
    # tau = max(mx - 1, d)
    mk("tau_b", "ANT_TAU_B", maxx(Src0 - One, Src1))
    # sparsemax threshold: relu(x - tau)
    mk("relusub", "ANT_RELUSUB", relu(Src0 - Src1))
    # z = max(a + b, eps)
    mk("addmax", "ANT_ADDMAX", maxx(Src0 + Src1, C0))
    # cos = a*b + 0.1
    mk("fmac", "ANT_FMA_C", Src0 * Src1 + C0)
    # scale = (a*21)*b
    mk("smul", "ANT_SMUL", (Src0 * C0) * Src1)
    # out = max(a*b, 0.001)
    mk("maxmul", "ANT_MAXMUL", maxx(Src0 * Src1, C0))
    return _OPS_CACHE


# --------------------------------------------------------------------------
# Bass program
# --------------------------------------------------------------------------
_PROG_CACHE: dict = {}


def _build_program(z_eps: float, scale_factor: float, B: int, T: int):
    """One SPMD program; every core runs the same code on its own shard."""
    ops = _custom_ops()
    # Bacc (not raw Bass): its compile() runs generate_event_semaphores,
    # which legalizes multi-event-sem waits the DVE/CTRL structs can't carry.
    nc = bacc.Bacc(
        "TRN2",
        target_bir_lowering=False,
        num_devices=N_CORES,
        dynamic_dma_scratch_size=8192,
    )
    e_pad = PDIM * B * T

    # Single fused input stream per tile (one DMA -> one DMA-sem wait per
    # consumer; the DVE TT struct only tolerates a single event-sem wait).
    # Per tile, per partition: [3B child | 3B parent | 12B gathered-table].
    xin_d = nc.dram_tensor("xin", [e_pad * 18], F32, kind="ExternalInput")
    out_d = nc.dram_tensor("alpha", [e_pad * 3], F32, kind="ExternalOutput")

    xin_v = xin_d[:].rearrange("(t p c) -> t p c", t=T, p=PDIM)
    out_v = out_d[:].rearrange("(t p c) -> t p c", t=T, p=PDIM)

    with TileContext(nc) as tc:
        with (
            tc.tile_pool(name="io", bufs=2) as iop,
            tc.tile_pool(name="scr", bufs=1) as sp,
        ):
            for t in range(T):
                xin = iop.tile([PDIM, 18 * B], F32, tag="xin", name=f"xin{t}")
                ot = iop.tile([PDIM, 3 * B], F32, tag="ot", name=f"ot{t}")
                nc.sync.dma_start(xin[:], xin_v[t])

                # Planar layout: 18 contiguous planes of B per partition:
                # [cp0 cp1 cp2 | pp0 pp1 pp2 | m00..m22 | b0 b1 b2]
                cp_all = xin[:, 0 : 3 * B]
                pp_all = xin[:, 3 * B : 6 * B]
                mrow = [xin[:, (6 + 3 * i) * B : (9 + 3 * i) * B] for i in range(3)]
                b_all = xin[:, 15 * B : 18 * B]

                # output stays planar (3 planes of B); host re-interleaves

                def pl(tag):
                    return sp.tile(
                        [PDIM, B], F32, tag=tag, name=f"{tag}{t}", bufs=2
                    )[:]

                V = nc.vector
                A = nc.scalar

                def cdve(op_key, out, in0, in1, s0=0.0, s1=0.0, imm2=0.0):
                    V._custom_dve(
                        ops[op_key], out=out, in0=in0, in1=in1, s0=s0, s1=s1, imm2=imm2
                    )

                # ww: 6 planes; first half doubles as w1, second as l_all.
                # Keeping them in ONE tile lets paired sum-chains run as
                # single strided-wide adds into separate 2-plane outputs.
                ww = sp.tile([PDIM, 6 * B], F32, tag="ww", name=f"ww{t}", bufs=2)[:]
                w1 = ww[:, 0 : 3 * B]
                l_all = ww[:, 3 * B : 6 * B]
                c_all = sp.tile([PDIM, 3 * B], F32, tag="call", name=f"call{t}")[:]
                p_all = sp.tile([PDIM, 3 * B], F32, tag="pall", name=f"pall{t}")[:]

                def bcast3(plane_ap):
                    """[P,B] plane broadcast to [P,3,B] via a 0-stride dim."""
                    return plane_ap.rearrange("p (u b) -> p u b", u=1).broadcast_to(
                        [PDIM, 3, B]
                    )

                def pair_sum3(src6, out2):
                    """out2[:, k*B+e] = sum_j src6[:, (3k+j)*B+e] for k=0,1."""
                    v = src6.rearrange("p (n b) -> p n b", b=B)
                    x = v[:, 0:6:3, :]
                    y = v[:, 1:6:3, :]
                    z = v[:, 2:6:3, :]
                    o = out2.rearrange("p (n b) -> p n b", b=B)
                    V.tensor_add(o, x, y)
                    V.tensor_add(o, o, z)

                # ---- c_raw = M[rel] @ child: rows 0,1 in one 6-plane mul
                # (cp broadcast across the two rows), row 2 separately.
                cp2 = cp_all.rearrange("p (u x) -> p u x", u=1).broadcast_to(
                    [PDIM, 2, 3 * B]
                )
                V.tensor_mul(
                    ww.rearrange("p (u x) -> p u x", x=3 * B),
                    xin[:, 6 * B : 12 * B].rearrange("p (u x) -> p u x", x=3 * B),
                    cp2,
                )
                pair_sum3(ww, c_all[:, 0 : 2 * B])  # c0|c1
                V.tensor_mul(w1, mrow[2], cp_all)
                c2 = c_all[:, 2 * B : 3 * B]
                V.tensor_add(c2, w1[:, 0:B], w1[:, B : 2 * B])
                V.tensor_add(c2, c2, w1[:, 2 * B : 3 * B])

                # ---- sparsemax (simplex projection, d=3):
                # tau = max(mx-1, (sm-mn-1)/2, (sm-1)/3); out = relu(x - tau)
                def sparsemax(x_all, out_all, tag):
                    x = [x_all[:, j * B : (j + 1) * B] for j in range(3)]
                    mx, mn, sm, dd = (
                        pl(f"mx{tag}"),
                        pl(f"mn{tag}"),
                        pl(f"sm{tag}"),
                        pl(f"dd{tag}"),
                    )
                    V.tensor_max(mx, x[0], x[1])
                    V.tensor_max(mx, mx, x[2])
                    V.tensor_tensor(mn, x[0], x[1], OP.min)
                    V.tensor_tensor(mn, mn, x[2], OP.min)
                    V.tensor_add(sm, x[0], x[1])
                    V.tensor_add(sm, sm, x[2])
                    cdve("tau_a", dd, sm, mn, s0=0.5, s1=1.0 / 3.0)
                    cdve("tau_b", dd, mx, dd)
                    ov = out_all.rearrange("p (n b) -> p n b", b=B)
                    xv = x_all.rearrange("p (n b) -> p n b", b=B)
                    cdve("relusub", ov, xv, bcast3(dd))
                    return mx, mn, sm, dd

                # c = sparsemax(c_raw); the reference's second application is
                # an exact no-op (projection idempotence) up to ~1e-7: skipped.
                lzs = sparsemax(c_all, c_all, "c")[0]  # mx plane recycled
                ncs = sparsemax(pp_all, p_all, "p")[0]  # mx plane recycled

                # ---- z = max(p + c, eps); entropy = ln(zs) - sum(z ln z)/zs
                # zq = [zs | szl] computed as one paired sum over ww
                w1v = w1.rearrange("p (n b) -> p n b", b=B)
                cvv = c_all.rearrange("p (n b) -> p n b", b=B)
                pvv = p_all.rearrange("p (n b) -> p n b", b=B)
                cdve("addmax", w1v, cvv, pvv, s0=z_eps)  # w1 <- z (wide)
                A.activation(l_all, w1, AF.Ln)  # wide ln
                V.tensor_mul(l_all, w1, l_all)  # l <- z*ln z (wide)
                zq = sp.tile([PDIM, 2 * B], F32, tag="zq", name=f"zq{t}", bufs=2)[:]
                pair_sum3(ww, zq)  # zq <- [zs | szl]
                zs = zq[:, 0:B]
                szl = zq[:, B : 2 * B]
                A.activation(lzs, zs, AF.Ln)
                A.activation(zs, lzs, AF.Exp, scale=-1.0)  # zs <- 1/zs
                V.tensor_mul(szl, szl, zs)
                V.tensor_sub(lzs, lzs, szl)  # lzs <- entropy

                # ---- cos = 0.1 + (p.c) / sqrt((p.p)*(c.c))
                V.tensor_mul(w1, p_all, c_all)  # w1 <- pc (wide)
                A.square(l_all, p_all)  # wide
                nq = sp.tile([PDIM, 2 * B], F32, tag="nq", name=f"nq{t}", bufs=2)[:]
                pair_sum3(ww, nq)  # nq <- [p.c | p.p]
                num = nq[:, 0:B]
                nps = nq[:, B : 2 * B]
                A.square(l_all, c_all)  # wide
                V.tensor_add(ncs, l_all[:, 0:B], l_all[:, B : 2 * B])
                V.tensor_add(ncs, ncs, l_all[:, 2 * B : 3 * B])
                # 1/(|p||c|) = exp(-(ln nps + ln ncs)/2)
                A.activation(nps, nps, AF.Ln)
                A.activation(ncs, ncs, AF.Ln)
                V.tensor_add(ncs, nps, ncs)
                A.activation(ncs, ncs, AF.Exp, scale=-0.5)
                cdve("fmac", num, num, ncs, s0=0.1)  # num <- cos

                # ---- alpha0 = p + b*(c-p) (wide, into w1)
                V.tensor_sub(w1, c_all, p_all)
                V.tensor_mul(w1, w1, b_all)
                V.tensor_add(w1, p_all, w1)

                # ---- scale = sf*cos/entropy; out = max(alpha0*scale, 1e-3)
                # 1/entropy = exp(-ln entropy)
                A.activation(lzs, lzs, AF.Ln)
                A.activation(lzs, lzs, AF.Exp, scale=-1.0)
                cdve("smul", num, num, lzs, s0=scale_factor)  # num <- scale
                cdve("maxmul", ot[:].rearrange("p (n b) -> p n b", b=B), w1v,
                     bcast3(num), s0=0.001)

                nc.sync.dma_start(out_v[t], ot[:])

    nc.compile()
    return nc


def _get_program(z_eps: float, scale_factor: float, B: int, T: int):
    key = (round(z_eps, 9), round(scale_factor, 9), B, T)
    if key not in _PROG_CACHE:
        _PROG_CACHE[key] = _build_program(z_eps, scale_factor, B, T)
    return _PROG_CACHE[key]


# --------------------------------------------------------------------------
# Host entry point
# --------------------------------------------------------------------------
_B = 656
_T = 12
E_PAD = PDIM * _B * _T  # 1,007,616 >= 1,000,000


def _fused_shard(
    child: np.ndarray,
    prnt: np.ndarray,
    g: np.ndarray,
    k: int,
    e: int,
    e_pad: int,
    B: int,
    T: int,
) -> np.ndarray:
    """Per-core fused input stream [T, 128, 18, B]: per tile & partition, 18
    contiguous planes of B: [cp0 cp1 cp2 | pp0 pp1 pp2 | m00..m22 | b0 b1 b2],
    flattened f32 (planar keeps every device access contiguous)."""

    def pad(a):
        sl = a[k * e : (k + 1) * e]
        out = np.zeros((e_pad, a.shape[1]), dtype=np.float32)
        out[: sl.shape[0]] = sl
        # [e_pad, w] -> [T, 128, B, w] -> planar [T, 128, w, B]
        return out.reshape(T, PDIM, B, -1).transpose(0, 1, 3, 2)

    return np.ascontiguousarray(
        np.concatenate([pad(child), pad(prnt), pad(g)], axis=2)
    ).reshape(-1)


def _run(inputs: dict, trace: bool = False):
    child = np.asarray(inputs["child_probs"], dtype=np.float32)
    prnt = np.asarray(inputs["prnt_probs"], dtype=np.float32)
    M = np.asarray(inputs["M"], dtype=np.float32)
    beta = np.asarray(inputs["beta"], dtype=np.float32)
    rels = np.asarray(inputs["rels"])
    z_eps = float(np.asarray(inputs["z_epsilon"]))
    sf = float(np.asarray(inputs["scale_factor"]))

    n = rels.shape[0]
    assert n % N_CORES == 0
    e = n // N_CORES
    assert e <= E_PAD

    t12 = np.concatenate([M.reshape(M.shape[0], 9), beta], axis=1).astype(np.float32)
    g = t12[rels]  # [N, 12]

    nc = _get_program(z_eps, sf, _B, _T)
    in_maps = [
        {"xin": _fused_shard(child, prnt, g, k, e, E_PAD, _B, _T)}
        for k in range(N_CORES)
    ]
    res = run_bass_kernel_spmd(nc, in_maps, core_ids=list(range(N_CORES)), trace=trace)
    # device emits planar [T, 128, 3, B]; re-interleave to [E, 3]
    outs = [
        r["alpha"]
        .reshape(_T, PDIM, 3, _B)
        .transpose(0, 1, 3, 2)
        .reshape(E_PAD, 3)[:e]
        for r in res.results
    ]
    return np.concatenate(outs, axis=0), res


def kernel(**inputs) -> np.ndarray:
    out, _ = _run(inputs)
    return out


def kernel_traced(**inputs):
    """Returns (output, BassKernelResults-with-profile) for test harnesses."""
    return _run(inputs, trace=True)



# revision 6
# speedup vs baseline: 1.3843x; 1.3843x over previous
"""AlphaModel (relation-gated message passing) Trainium2 kernel, v2.

Strategy (pure data parallel, per sharding hint):
  - Shard the 8M edges across 8 NeuronCores (1M each).
  - Host SORTS each core's edges by relation id and packs them into
    (tile, partition) cells so that every SBUF partition row processes
    edges of a single relation.  M[rel] / beta[rel] then enter the
    device as per-partition scalar vectors [128,1] (fp32), so the
    relation-gated matvec and beta-mix become tensor_scalar ops (which
    run in the DVE 2x/4x perf modes) instead of streamed per-edge
    tables.  DMA drops from 84B/edge (f32 fused stream) to ~18B/edge
    (fp16 child/parent in, fp16 alpha out).
  - Compute is fp16 end-to-end on-chip (tolerance is 2e-2 rel; fp16
    keeps per-op error ~5e-4).  tensor_tensor ops hit the 2x_1p DVE
    mode, tensor_scalar ops the 4x_2p mode; the only custom DVE op kept
    is the sparsemax tau candidate (ANT_TAU_A), everything else maps
    cheaper onto standard 2x/4x ops.  Ln/Exp/Square run on the Scalar
    (ACT) engine; a slice of the sparsemax max/min/add tree runs on
    GPSIMD to overlap all three elementwise engines.
  - sparsemax (d=3) via simplex projection:
      tau = max(mx-1, (sm-mn-1)/2, (sm-1)/3);  out = relu(x - tau)
    The reference's second sparsemax application is an exact no-op
    (idempotent projection) and is skipped.

Output: alpha [8M, 3] float32 (device emits fp16, host widens).
"""

import sys

if "/opt/trn_rl_repo" not in sys.path:
    sys.path.insert(0, "/opt/trn_rl_repo")

import numpy as np

import concourse.bacc as bacc
import concourse.mybir as mybir
from concourse.bass_utils import run_bass_kernel_spmd
from concourse.tile import TileContext

N_CORES = 8
PDIM = 128
N_RELS = 64
B = 896  # edges per (tile, partition) cell

AF = mybir.ActivationFunctionType
OP = mybir.AluOpType
F16 = mybir.dt.float16
F32 = mybir.dt.float32

# --------------------------------------------------------------------------
# Custom fused DVE op (same registration machinery as production dve_ops).
# --------------------------------------------------------------------------
_OPS_CACHE: dict = {}


def _custom_ops():
    if _OPS_CACHE:
        return _OPS_CACHE
    from concourse import dve_ops
    from concourse.dve_ops import DveOp, OPS, _SUB_OPCODE_FOR_NAME
    from concourse.dve_spec import (
        C0,
        C1,
        One,
        Spec,
        Src0,
        Src1,
        _has_src1,
        lower,
        maxx,
    )
    from concourse.dve_uop import DveOpSpec

    existing = {op.name: op for op in OPS}

    def mk(key, name, body):
        if name in existing:
            _OPS_CACHE[key] = existing[name]
            return
        if name not in _SUB_OPCODE_FOR_NAME:
            row = max(_SUB_OPCODE_FOR_NAME.values()) + 1
            assert row < 0x20, "custom DVE opcode rows exhausted"
            _SUB_OPCODE_FOR_NAME[name] = row
        spec = Spec(body=body)
        shas = {}
        for ver in ("v3", "v4"):
            uops = lower(spec, ver=ver)
            s = DveOpSpec(
                name=name,
                opcode=_SUB_OPCODE_FOR_NAME[name],
                uops=uops,
                rd1_en=_has_src1(spec),
            )
            shas[ver] = s.sha(ver)
        op = DveOp(name, spec, subdim=False, uops_sha=shas)
        OPS.append(op)
        dve_ops.CUSTOM_DVE_SPECS[name] = spec
        _OPS_CACHE[key] = op

    # tau candidates: max((sm - mn - 1)*0.5, (sm - 1)/3);  in0=sm, in1=mn
    mk("tau_a", "ANT_TAU_A", maxx((Src0 - Src1 - One) * C0, (Src0 - One) * C1))
    return _OPS_CACHE


# --------------------------------------------------------------------------
# Bass program
# --------------------------------------------------------------------------
_PROG_CACHE: dict = {}


def _build_program(z_eps: float, scale_factor: float, T: int):
    ops = _custom_ops()
    nc = bacc.Bacc(
        "TRN2",
        target_bir_lowering=False,
        num_devices=N_CORES,
        dynamic_dma_scratch_size=8192,
    )
    ln21 = float(np.log(scale_factor))

    # Input stream per tile/partition: [ch0 ch1 ch2 | pp0 pp1 pp2] fp16
    xin_d = nc.dram_tensor("xin", [T * PDIM * 6 * B], F16, kind="ExternalInput")
    # Per (tile, partition) scalars: M00..M22, b0..b2, (1-b)0..(1-b)2, pad
    scl_d = nc.dram_tensor("scl", [T * PDIM * 16], F32, kind="ExternalInput")
    out_d = nc.dram_tensor("alpha", [T * PDIM * 3 * B], F16, kind="ExternalOutput")

    xin_v = xin_d[:].rearrange("(t p c) -> t p c", t=T, p=PDIM)
    scl_v = scl_d[:].rearrange("(t p c) -> t p c", t=T, p=PDIM)
    out_v = out_d[:].rearrange("(t p c) -> t p c", t=T, p=PDIM)

    V = nc.vector
    G = nc.gpsimd
    A = nc.scalar

    with TileContext(nc) as tc:
        with (
            tc.tile_pool(name="io", bufs=2) as iop,
            tc.tile_pool(name="scr", bufs=2) as sp,
        ):
            for t in range(T):
                xin = iop.tile([PDIM, 6 * B], F16, tag="xin", name=f"xin{t}")
                scl = iop.tile([PDIM, 16], F32, tag="scl", name=f"scl{t}")
                ot = iop.tile([PDIM, 3 * B], F16, tag="ot", name=f"ot{t}")
                nc.sync.dma_start(xin[:], xin_v[t])
                nc.sync.dma_start(scl[:], scl_v[t])

                X = xin[:, 0 : 3 * B]  # child planes
                Pr = xin[:, 3 * B : 6 * B]  # parent (raw) planes

                def wide(tag):
                    return sp.tile([PDIM, 3 * B], F16, tag=tag, name=f"{tag}{t}")[:]

                def pl(tag):
                    return sp.tile([PDIM, B], F16, tag=tag, name=f"{tag}{t}")[:]

                def p3(x, i):
                    return x[:, i * B : (i + 1) * B]

                def sv(j):
                    return scl[:, j : j + 1]

                CR = wide("CR")  # raw matvec out; later reused as p*c
                Cc = wide("Cc")  # sparsemax(c)
                Pp = wide("Pp")  # sparsemax(p)
                Y = wide("Y")  # z (unnormalized); later alpha0
                LN = wide("LN")  # ln z -> z ln z; later beta-mix scratch
                U1 = wide("U1")  # p^2 / c^2 scratch
                U2 = wide("U2")

                # ---- c_raw_i = sum_j M_ij * ch_j  (per-partition M scalars)
                tmp = pl("mvt")
                for i in range(3):
                    V.tensor_scalar(p3(CR, i), p3(X, 0), sv(3 * i + 0), None, OP.mult)
                    V.tensor_scalar(tmp, p3(X, 1), sv(3 * i + 1), None, OP.mult)
                    V.tensor_add(p3(CR, i), p3(CR, i), tmp)
                    V.tensor_scalar(tmp, p3(X, 2), sv(3 * i + 2), None, OP.mult)
                    V.tensor_add(p3(CR, i), p3(CR, i), tmp)

                # ---- sparsemax: dst = relu(src - tau)
                def sparsemax(src, dst, tag):
                    mx = pl(f"mx{tag}")
                    mn = pl(f"mn{tag}")
                    sm = pl(f"sm{tag}")
                    ta = pl(f"ta{tag}")
                    x0, x1, x2 = p3(src, 0), p3(src, 1), p3(src, 2)
                    V.tensor_max(mx, x0, x1)
                    V.tensor_tensor(mn, x0, x1, OP.min)
                    V.tensor_add(sm, x0, x1)
                    V.tensor_max(mx, mx, x2)
                    V.tensor_tensor(mn, mn, x2, OP.min)
                    V.tensor_add(sm, sm, x2)
                    V._custom_dve(
                        ops["tau_a"], out=ta, in0=sm, in1=mn, s0=0.5, s1=1.0 / 3.0
                    )
                    V.tensor_scalar(mx, mx, -1.0, None, OP.add)  # mx-1 (4x)
                    V.tensor_max(ta, ta, mx)  # tau
                    for i in range(3):
                        V.tensor_sub(p3(dst, i), p3(src, i), ta)
                    V.tensor_scalar(dst, dst, 0.0, None, OP.max)  # relu (4x wide)

                sparsemax(CR, Cc, "c")
                sparsemax(Pr, Pp, "p")

                # ---- z = max(p + c, eps); l = z ln z
                V.tensor_add(Y, Pp, Cc)
                V.tensor_scalar(Y, Y, float(z_eps), None, OP.max)
                A.activation(LN, Y, AF.Ln)
                V.tensor_mul(LN, Y, LN)
                zs = pl("zs")
                szl = pl("szl")
                V.tensor_add(zs, p3(Y, 0), p3(Y, 1))
                V.tensor_add(zs, zs, p3(Y, 2))
                V.tensor_add(szl, p3(LN, 0), p3(LN, 1))
                V.tensor_add(szl, szl, p3(LN, 2))

                # ---- cosine pieces: pc, |p|^2, |c|^2
                pc = pl("pc")
                pps = pl("pps")
                ccs = pl("ccs")
                V.tensor_mul(CR, Pp, Cc)  # reuse CR as p*c (wide)
                V.tensor_add(pc, p3(CR, 0), p3(CR, 1))
                V.tensor_add(pc, pc, p3(CR, 2))
                A.square(U1, Pp)
                V.tensor_add(pps, p3(U1, 0), p3(U1, 1))
                V.tensor_add(pps, pps, p3(U1, 2))
                A.square(U2, Cc)
                V.tensor_add(ccs, p3(U2, 0), p3(U2, 1))
                V.tensor_add(ccs, ccs, p3(U2, 2))

                # ---- entropy = ln zs - szl/zs ; cos = 0.1 + pc/sqrt(pps*ccs)
                lzs = pl("lzs")
                izs = pl("izs")
                ent = pl("ent")
                irt = pl("irt")
                ie = pl("ie")
                A.activation(lzs, zs, AF.Ln)
                A.activation(izs, lzs, AF.Exp, scale=-1.0)  # 1/zs
                V.tensor_mul(szl, szl, izs)
                V.tensor_sub(ent, lzs, szl)
                V.tensor_mul(pps, pps, ccs)  # nn = pps*ccs
                A.activation(irt, pps, AF.Ln)
                A.activation(irt, irt, AF.Exp, scale=-0.5)  # 1/sqrt(nn)
                V.tensor_mul(pc, pc, irt)
                V.tensor_scalar(pc, pc, 0.1, None, OP.add)  # cos (4x)
                A.activation(lzs, ent, AF.Ln)  # reuse lzs
                A.activation(ie, lzs, AF.Exp, scale=-1.0)  # 1/ent
                sc = pl("sc")
                V.tensor_mul(sc, pc, ie)  # cos/ent  (x21 folded into final op)

                # ---- alpha = max(scale * (b*c + (1-b)*p), 0.001)
                for i in range(3):
                    V.tensor_scalar(p3(Y, i), p3(Cc, i), sv(9 + i), None, OP.mult)
                    V.tensor_scalar(p3(LN, i), p3(Pp, i), sv(12 + i), None, OP.mult)
                V.tensor_add(Y, Y, LN)  # alpha0 (wide)
                for i in range(3):
                    V.tensor_mul(p3(Y, i), p3(Y, i), sc)
                # alpha = max(21 * alpha0*(cos/ent), 0.001) - one fused 4x op
                V.tensor_scalar(ot[:], Y, float(scale_factor), 0.001, OP.mult, OP.max)

                nc.sync.dma_start(out_v[t], ot[:])

    nc.compile()
    return nc


def _get_program(z_eps: float, scale_factor: float, T: int):
    key = (round(z_eps, 9), round(scale_factor, 9), T)
    if key not in _PROG_CACHE:
        _PROG_CACHE[key] = _build_program(z_eps, scale_factor, T)
    return _PROG_CACHE[key]


# --------------------------------------------------------------------------
# Host-side pack/unpack
# --------------------------------------------------------------------------
def _cells_for(counts: np.ndarray):
    """Cell table (rel, start-within-rel) chunks of B for one core."""
    nch = (counts + B - 1) // B  # chunks per rel
    cell_rel = np.repeat(np.arange(N_RELS, dtype=np.int64), nch)
    within = np.concatenate(
        [np.arange(n, dtype=np.int64) * B for n in nch]
    ) if nch.sum() else np.zeros(0, np.int64)
    return cell_rel, within


def _pack_core(ch, pp, rels, t15, k, e, T):
    """Returns (xin_f16, scl_f32, order, gi, valid, ncells)."""
    sl = slice(k * e, (k + 1) * e)
    r = rels[sl]
    order = np.argsort(r, kind="stable")
    counts = np.bincount(r, minlength=N_RELS)
    cell_rel, within = _cells_for(counts)
    ncells = cell_rel.shape[0]
    ncap = T * PDIM
    assert ncells <= ncap, (ncells, ncap)
    rel_starts = np.concatenate([[0], np.cumsum(counts)[:-1]])
    cell_start = rel_starts[cell_rel] + within
    cell_len = np.minimum(counts[cell_rel] - within, B)
    # pad cell table to capacity
    pad = ncap - ncells
    if pad:
        cell_rel = np.concatenate([cell_rel, np.zeros(pad, np.int64)])
        cell_start = np.concatenate([cell_start, np.zeros(pad, np.int64)])
        cell_len = np.concatenate([cell_len, np.zeros(pad, np.int64)])
    gi = cell_start[:, None] + np.arange(B, dtype=np.int64)[None, :]
    valid = np.arange(B, dtype=np.int64)[None, :] < cell_len[:, None]
    gi = np.where(valid, gi, 0)

    def pack(a):
        s = np.ascontiguousarray(a[sl], dtype=np.float32)[order]  # [e, 3]
        cells = s[gi]  # [ncap, B, 3]
        cells *= valid[..., None]
        # -> [T, 128, 3, B] planar fp16
        return (
            cells.transpose(0, 2, 1)
            .reshape(T, PDIM, 3 * B)
            .astype(np.float16)
        )

    chp = pack(ch)
    ppp = pack(pp)
    xin = np.concatenate([chp, ppp], axis=2).reshape(-1)
    scl = np.zeros((ncap, 16), dtype=np.float32)
    scl[:, :15] = t15[cell_rel]
    return xin, scl.reshape(-1), order, gi, valid


def _unpack_core(out_f16, order, gi, valid, e, T):
    cells = (
        out_f16.reshape(T * PDIM, 3, B).transpose(0, 2, 1).astype(np.float32)
    )  # [ncap, B, 3]
    res_sorted = np.empty((e, 3), dtype=np.float32)
    res_sorted[gi[valid]] = cells[valid]
    res = np.empty((e, 3), dtype=np.float32)
    res[order] = res_sorted
    return res


def _run(inputs: dict, trace: bool = False):
    ch = np.asarray(inputs["child_probs"], dtype=np.float32)
    pp = np.asarray(inputs["prnt_probs"], dtype=np.float32)
    M = np.asarray(inputs["M"], dtype=np.float32)
    beta = np.asarray(inputs["beta"], dtype=np.float32)
    rels = np.asarray(inputs["rels"]).astype(np.int64)
    z_eps = float(np.asarray(inputs["z_epsilon"]))
    sf = float(np.asarray(inputs["scale_factor"]))

    n = rels.shape[0]
    assert n % N_CORES == 0
    e = n // N_CORES

    t15 = np.concatenate(
        [M.reshape(N_RELS, 9), beta, 1.0 - beta], axis=1
    ).astype(np.float32)

    # T: tiles needed to fit every core's cell count
    max_cells = 0
    for k in range(N_CORES):
        counts = np.bincount(rels[k * e : (k + 1) * e], minlength=N_RELS)
        max_cells = max(max_cells, int(((counts + B - 1) // B).sum()))
    T = max(1, -(-max_cells // PDIM))

    packs = [_pack_core(ch, pp, rels, t15, k, e, T) for k in range(N_CORES)]
    nc = _get_program(z_eps, sf, T)
    in_maps = [{"xin": p[0], "scl": p[1]} for p in packs]
    res = run_bass_kernel_spmd(nc, in_maps, core_ids=list(range(N_CORES)), trace=trace)
    outs = [
        _unpack_core(res.results[k]["alpha"], packs[k][2], packs[k][3], packs[k][4], e, T)
        for k in range(N_CORES)
    ]
    return np.concatenate(outs, axis=0), res


def kernel(**inputs) -> np.ndarray:
    out, _ = _run(inputs)
    return out


def kernel_traced(**inputs):
    """Returns (output, BassKernelResults-with-profile) for test harnesses."""
    return _run(inputs, trace=True)


# revision 10
# speedup vs baseline: 1.4451x; 1.0439x over previous
"""AlphaModel (relation-gated message passing) Trainium2 kernel, v3.

Strategy (pure data parallel, per sharding hint):
  - Shard the 8M edges across 8 NeuronCores (1M each).
  - Host SORTS each core's edges by relation id and packs them into
    (tile, partition) cells so that every SBUF partition row processes
    edges of a single relation.  M[rel] / beta[rel] then enter the
    device as per-partition scalar vectors [128,1] (fp32), so the
    relation-gated matvec and beta-mix become tensor_scalar ops (DVE
    2x/4x perf modes) instead of streamed per-edge tables.
  - fp16 end-to-end on-chip (2e-2 rel tolerance).  tensor_tensor ops
    hit the 2x_1p DVE mode, tensor_scalar the 4x_2p mode.
  - The c- and p-branch sparsemaxes are FUSED: both live in one wide
    [128, 6B] tile ([c0 c1 c2 | p0 p1 p2]) and the whole max/min/sum
    tree + tau runs as [128, 2, B] strided ops covering both branches.
  - Activation-table thrash eliminated: only Ln / Square / one Exp on
    ACT (grouped), reciprocals on the DVE (nc.vector.reciprocal).
  - sparsemax (d=3) via simplex projection:
      tau = max(mx-1, (sm-mn-1)/2, (sm-1)/3);  out = relu(x - tau)
    The second sparsemax application in the reference is an exact no-op
    (idempotent projection) and is skipped.

Output: alpha [8M, 3] float32 (device emits fp16, host widens).
"""

import sys

if "/opt/trn_rl_repo" not in sys.path:
    sys.path.insert(0, "/opt/trn_rl_repo")

import numpy as np

import concourse.bacc as bacc
import concourse.mybir as mybir
from concourse.bass_utils import run_bass_kernel_spmd
from concourse.tile import TileContext

N_CORES = 8
PDIM = 128
N_RELS = 64
B = 1152  # edges per (tile, partition) cell

AF = mybir.ActivationFunctionType
OP = mybir.AluOpType
F16 = mybir.dt.float16
F32 = mybir.dt.float32

# --------------------------------------------------------------------------
# Custom fused DVE op (same registration machinery as production dve_ops).
# --------------------------------------------------------------------------
_OPS_CACHE: dict = {}


def _custom_ops():
    if _OPS_CACHE:
        return _OPS_CACHE
    from concourse import dve_ops
    from concourse.dve_ops import DveOp, OPS, _SUB_OPCODE_FOR_NAME
    from concourse.dve_spec import (
        C0,
        C1,
        One,
        Spec,
        Src0,
        Src1,
        _has_src1,
        lower,
        maxx,
    )
    from concourse.dve_uop import DveOpSpec

    existing = {op.name: op for op in OPS}

    def mk(key, name, body):
        if name in existing:
            _OPS_CACHE[key] = existing[name]
            return
        if name not in _SUB_OPCODE_FOR_NAME:
            row = max(_SUB_OPCODE_FOR_NAME.values()) + 1
            assert row < 0x20, "custom DVE opcode rows exhausted"
            _SUB_OPCODE_FOR_NAME[name] = row
        spec = Spec(body=body)
        shas = {}
        for ver in ("v3", "v4"):
            uops = lower(spec, ver=ver)
            s = DveOpSpec(
                name=name,
                opcode=_SUB_OPCODE_FOR_NAME[name],
                uops=uops,
                rd1_en=_has_src1(spec),
            )
            shas[ver] = s.sha(ver)
        op = DveOp(name, spec, subdim=False, uops_sha=shas)
        OPS.append(op)
        dve_ops.CUSTOM_DVE_SPECS[name] = spec
        _OPS_CACHE[key] = op

    # tau candidates: max((sm - mn - 1)*0.5, (sm - 1)/3);  in0=sm, in1=mn
    mk("tau_a", "ANT_TAU_A", maxx((Src0 - Src1 - One) * C0, (Src0 - One) * C1))
    return _OPS_CACHE


# --------------------------------------------------------------------------
# Bass program
# --------------------------------------------------------------------------
_PROG_CACHE: dict = {}


def _build_program(z_eps: float, scale_factor: float, T: int):
    ops = _custom_ops()
    nc = bacc.Bacc(
        "TRN2",
        target_bir_lowering=False,
        num_devices=N_CORES,
        dynamic_dma_scratch_size=8192,
    )

    # Input stream per tile/partition: [ch0 ch1 ch2 | pp0 pp1 pp2] fp16
    xin_d = nc.dram_tensor("xin", [T * PDIM * 6 * B], F16, kind="ExternalInput")
    # Per (tile, partition) scalars: M00..M22, b0..b2, (1-b)0..(1-b)2, pad
    scl_d = nc.dram_tensor("scl", [T * PDIM * 16], F32, kind="ExternalInput")
    out_d = nc.dram_tensor("alpha", [T * PDIM * 3 * B], F16, kind="ExternalOutput")

    xin_v = xin_d[:].rearrange("(t p c) -> t p c", t=T, p=PDIM)
    scl_v = scl_d[:].rearrange("(t p c) -> t p c", t=T, p=PDIM)
    out_v = out_d[:].rearrange("(t p c) -> t p c", t=T, p=PDIM)

    V = nc.vector
    A = nc.scalar

    with TileContext(nc) as tc:
        with (
            nc.allow_low_precision(reason="fp16 pipeline; 2e-2 rel tolerance"),
            tc.tile_pool(name="io", bufs=2) as iop,
            tc.tile_pool(name="scr", bufs=2) as sp,
        ):
            for t in range(T):
                xin = iop.tile([PDIM, 6 * B], F16, tag="xin", name=f"xin{t}")
                scl = iop.tile([PDIM, 16], F32, tag="scl", name=f"scl{t}")
                ot = iop.tile([PDIM, 3 * B], F16, tag="ot", name=f"ot{t}")
                nc.sync.dma_start(xin[:], xin_v[t])
                nc.sync.dma_start(scl[:], scl_v[t])

                X = xin[:, 0 : 3 * B]  # child planes
                Pr = xin[:, 3 * B : 6 * B]  # parent (raw) planes

                def w6(tag, bufs=2):
                    return sp.tile(
                        [PDIM, 6 * B], F16, tag=tag, name=f"{tag}{t}", bufs=bufs
                    )[:]

                def w3(tag, bufs=1):
                    return sp.tile(
                        [PDIM, 3 * B], F16, tag=tag, name=f"{tag}{t}", bufs=bufs
                    )[:]

                def w2(tag, bufs=2):
                    return sp.tile(
                        [PDIM, 2 * B], F16, tag=tag, name=f"{tag}{t}", bufs=bufs
                    )[:]

                def pl(tag, bufs=1):
                    return sp.tile(
                        [PDIM, B], F16, tag=tag, name=f"{tag}{t}", bufs=bufs
                    )[:]

                def p3(x, i):
                    return x[:, i * B : (i + 1) * B]

                def sv(j):
                    return scl[:, j : j + 1]

                W = w6("W")  # [c_raw | p_raw] then later scratch
                SPX = w6("SPX")  # [c | p] sparsemax outputs
                YL = w6("YL", bufs=1)  # [y | y ln y]
                W9 = sp.tile([PDIM, 9 * B], F16, tag="W9", name=f"W9{t}", bufs=1)[:]

                CRw = W[:, 0 : 3 * B]
                # ---- c_raw_i = sum_j M_ij * ch_j  (per-partition M scalars)
                tmp = pl("mvt")
                for i in range(3):
                    V.tensor_scalar(p3(CRw, i), p3(X, 0), sv(3 * i), None, OP.mult)
                    V.tensor_scalar(tmp, p3(X, 1), sv(3 * i + 1), None, OP.mult)
                    V.tensor_add(p3(CRw, i), p3(CRw, i), tmp)
                    V.tensor_scalar(tmp, p3(X, 2), sv(3 * i + 2), None, OP.mult)
                    V.tensor_add(p3(CRw, i), p3(CRw, i), tmp)
                # copy p_raw next to c_raw (4x tensor_copy)
                V.tensor_copy(W[:, 3 * B : 6 * B], Pr)

                # ---- fused two-branch sparsemax: SPX = relu(W - tau)
                # strided views: component j of both branches = [128, 2, B]
                Wv = W.rearrange("p (u b) -> p u b", b=B)  # u = 6

                def comp(x6, j):
                    v = x6.rearrange("p (u b) -> p u b", b=B)
                    return v[:, j : j + 4 : 3, :]  # planes {j, j+3}

                mx = w2("mx")
                mn = w2("mn")
                sm = w2("sm")
                ta = w2("ta")
                mx2 = mx.rearrange("p (u b) -> p u b", b=B)
                mn2 = mn.rearrange("p (u b) -> p u b", b=B)
                sm2 = sm.rearrange("p (u b) -> p u b", b=B)
                ta2 = ta.rearrange("p (u b) -> p u b", b=B)
                x0, x1, x2 = comp(W, 0), comp(W, 1), comp(W, 2)
                V.tensor_max(mx2, x0, x1)
                V.tensor_tensor(mn2, x0, x1, OP.min)
                V.tensor_add(sm2, x0, x1)
                V.tensor_max(mx2, mx2, x2)
                V.tensor_tensor(mn2, mn2, x2, OP.min)
                V.tensor_add(sm2, sm2, x2)
                V._custom_dve(ops["tau_a"], out=ta, in0=sm, in1=mn, s0=0.5, s1=1.0 / 3.0)
                V.tensor_scalar(mx, mx, -1.0, None, OP.add)  # mx-1 (4x)
                V.tensor_max(ta, ta, mx)  # tau for both branches
                for j in range(3):
                    V.tensor_sub(comp(SPX, j), comp(W, j), ta2)
                V.tensor_scalar(SPX, SPX, 0.0, None, OP.max)  # relu (4x wide)

                Cc = SPX[:, 0 : 3 * B]
                Pp = SPX[:, 3 * B : 6 * B]

                # ---- y = max(p + c, eps); l = y ln y
                Y = YL[:, 0 : 3 * B]
                LNp = YL[:, 3 * B : 6 * B]
                V.tensor_add(Y, Pp, Cc)
                V.tensor_scalar(Y, Y, float(z_eps), None, OP.max)
                A.activation(LNp, Y, AF.Ln)
                V.tensor_mul(LNp, Y, LNp)
                # zq = [zs | szl] via strided pair-sums over YL
                zq = w2("zq", bufs=1)
                zq2 = zq.rearrange("p (u b) -> p u b", b=B)
                YLv = YL.rearrange("p (u b) -> p u b", b=B)
                V.tensor_add(zq2, YLv[:, 0:6:3, :], YLv[:, 1:6:3, :])
                V.tensor_add(zq2, zq2, YLv[:, 2:6:3, :])
                zs = zq[:, 0:B]
                szl = zq[:, B : 2 * B]

                # ---- cosine pieces: W9 = [p*c | p^2 | c^2], pq = sums
                V.tensor_mul(W9[:, 0 : 3 * B], Pp, Cc)
                A.square(W9[:, 3 * B : 6 * B], Pp)
                A.square(W9[:, 6 * B : 9 * B], Cc)
                pq = w3("pq")
                pq3 = pq.rearrange("p (u b) -> p u b", b=B)
                W9v = W9.rearrange("p (u b) -> p u b", b=B)
                V.tensor_add(pq3, W9v[:, 0:9:3, :], W9v[:, 1:9:3, :])
                V.tensor_add(pq3, pq3, W9v[:, 2:9:3, :])
                pc = pq[:, 0:B]
                pps = pq[:, B : 2 * B]
                ccs = pq[:, 2 * B : 3 * B]

                # ---- entropy = ln zs - szl/zs ; cos = 0.1 + pc/sqrt(pps*ccs)
                lzs = pl("lzs")
                izs = pl("izs")
                ent = pl("ent")
                nn = pl("nn")
                irt = pl("irt")
                ie = pl("ie")
                sc = pl("sc")
                V.tensor_mul(nn, pps, ccs)
                A.activation(lzs, zs, AF.Ln)
                A.activation(irt, nn, AF.Ln)  # ln(nn), same ACT table as above
                V.reciprocal(izs, zs)
                V.tensor_mul(szl, szl, izs)
                V.tensor_sub(ent, lzs, szl)
                A.activation(irt, irt, AF.Exp, scale=-0.5)  # 1/sqrt(nn)
                V.reciprocal(ie, ent)
                V.tensor_mul(pc, pc, irt)
                V.tensor_scalar(pc, pc, 0.1, None, OP.add)  # cos (4x)
                V.tensor_mul(sc, pc, ie)  # cos/ent  (x21 folded into final op)

                # ---- alpha = max(21 * (cos/ent) * (b*c + (1-b)*p), 0.001)
                for i in range(3):
                    V.tensor_scalar(p3(Y, i), p3(Cc, i), sv(9 + i), None, OP.mult)
                    V.tensor_scalar(p3(LNp, i), p3(Pp, i), sv(12 + i), None, OP.mult)
                V.tensor_add(Y, Y, LNp)  # alpha0 (wide)
                for i in range(3):
                    V.tensor_mul(p3(Y, i), p3(Y, i), sc)
                V.tensor_scalar(ot[:], Y, float(scale_factor), 0.001, OP.mult, OP.max)

                nc.sync.dma_start(out_v[t], ot[:])

    nc.compile()
    return nc


def _get_program(z_eps: float, scale_factor: float, T: int):
    key = (round(z_eps, 9), round(scale_factor, 9), T)
    if key not in _PROG_CACHE:
        _PROG_CACHE[key] = _build_program(z_eps, scale_factor, T)
    return _PROG_CACHE[key]


# --------------------------------------------------------------------------
# Host-side pack/unpack
# --------------------------------------------------------------------------
def _pack_core(ch, pp, rels, t15, k, e, T):
    """Returns (xin_f16, scl_f32, order, gi, valid)."""
    sl = slice(k * e, (k + 1) * e)
    r = rels[sl]
    order = np.argsort(r, kind="stable")
    counts = np.bincount(r, minlength=N_RELS)
    nch = (counts + B - 1) // B
    cell_rel = np.repeat(np.arange(N_RELS, dtype=np.int64), nch)
    within = np.concatenate([np.arange(n, dtype=np.int64) * B for n in nch])
    ncells = cell_rel.shape[0]
    ncap = T * PDIM
    assert ncells <= ncap, (ncells, ncap)
    rel_starts = np.concatenate([[0], np.cumsum(counts)[:-1]])
    cell_start = rel_starts[cell_rel] + within
    cell_len = np.minimum(counts[cell_rel] - within, B)
    pad = ncap - ncells
    if pad:
        cell_rel = np.concatenate([cell_rel, np.zeros(pad, np.int64)])
        cell_start = np.concatenate([cell_start, np.zeros(pad, np.int64)])
        cell_len = np.concatenate([cell_len, np.zeros(pad, np.int64)])
    gi = cell_start[:, None] + np.arange(B, dtype=np.int64)[None, :]
    valid = np.arange(B, dtype=np.int64)[None, :] < cell_len[:, None]
    gi = np.where(valid, gi, 0)

    def pack(a):
        s = np.ascontiguousarray(a[sl], dtype=np.float32)[order]  # [e, 3]
        cells = s[gi]  # [ncap, B, 3]
        cells *= valid[..., None]
        return cells.transpose(0, 2, 1).reshape(T, PDIM, 3 * B).astype(np.float16)

    xin = np.concatenate([pack(ch), pack(pp)], axis=2).reshape(-1)
    scl = np.zeros((ncap, 16), dtype=np.float32)
    scl[:, :15] = t15[cell_rel]
    return xin, scl.reshape(-1), order, gi, valid


def _unpack_core(out_f16, order, gi, valid, e, T):
    cells = out_f16.reshape(T * PDIM, 3, B).transpose(0, 2, 1).astype(np.float32)
    res_sorted = np.empty((e, 3), dtype=np.float32)
    res_sorted[gi[valid]] = cells[valid]
    res = np.empty((e, 3), dtype=np.float32)
    res[order] = res_sorted
    return res


def _run(inputs: dict, trace: bool = False):
    ch = np.asarray(inputs["child_probs"], dtype=np.float32)
    pp = np.asarray(inputs["prnt_probs"], dtype=np.float32)
    M = np.asarray(inputs["M"], dtype=np.float32)
    beta = np.asarray(inputs["beta"], dtype=np.float32)
    rels = np.asarray(inputs["rels"]).astype(np.int64)
    z_eps = float(np.asarray(inputs["z_epsilon"]))
    sf = float(np.asarray(inputs["scale_factor"]))

    n = rels.shape[0]
    assert n % N_CORES == 0
    e = n // N_CORES

    t15 = np.concatenate(
        [M.reshape(N_RELS, 9), beta, 1.0 - beta], axis=1
    ).astype(np.float32)

    max_cells = 0
    for k in range(N_CORES):
        counts = np.bincount(rels[k * e : (k + 1) * e], minlength=N_RELS)
        max_cells = max(max_cells, int(((counts + B - 1) // B).sum()))
    T = max(1, -(-max_cells // PDIM))

    packs = [_pack_core(ch, pp, rels, t15, k, e, T) for k in range(N_CORES)]
    nc = _get_program(z_eps, sf, T)
    in_maps = [{"xin": p[0], "scl": p[1]} for p in packs]
    res = run_bass_kernel_spmd(nc, in_maps, core_ids=list(range(N_CORES)), trace=trace)
    outs = [
        _unpack_core(res.results[k]["alpha"], packs[k][2], packs[k][3], packs[k][4], e, T)
        for k in range(N_CORES)
    ]
    return np.concatenate(outs, axis=0), res


def kernel(**inputs) -> np.ndarray:
    out, _ = _run(inputs)
    return out


def kernel_traced(**inputs):
    """Returns (output, BassKernelResults-with-profile) for test harnesses."""
    return _run(inputs, trace=True)


# revision 16
# speedup vs baseline: 1.7532x; 1.2132x over previous
"""AlphaModel (relation-gated message passing) Trainium2 kernel, v3.

Strategy (pure data parallel, per sharding hint):
  - Shard the 8M edges across 8 NeuronCores (1M each).
  - Host SORTS each core's edges by relation id and packs them into
    (tile, partition) cells so that every SBUF partition row processes
    edges of a single relation.  M[rel] / beta[rel] then enter the
    device as per-partition scalar vectors [128,1] (fp32), so the
    relation-gated matvec and beta-mix become tensor_scalar ops (DVE
    2x/4x perf modes) instead of streamed per-edge tables.
  - fp16 end-to-end on-chip (2e-2 rel tolerance).  tensor_tensor ops
    hit the 2x_1p DVE mode, tensor_scalar the 4x_2p mode.
  - The c- and p-branch sparsemaxes are FUSED: both live in one wide
    [128, 6B] tile ([c0 c1 c2 | p0 p1 p2]) and the whole max/min/sum
    tree + tau runs as [128, 2, B] strided ops covering both branches.
  - Activation-table thrash eliminated: only Ln / Square / one Exp on
    ACT (grouped), reciprocals on the DVE (nc.vector.reciprocal).
  - sparsemax (d=3) via simplex projection:
      tau = max(mx-1, (sm-mn-1)/2, (sm-1)/3);  out = relu(x - tau)
    The second sparsemax application in the reference is an exact no-op
    (idempotent projection) and is skipped.

Output: alpha [8M, 3] float32 (device emits fp16, host widens).
"""

import sys

if "/opt/trn_rl_repo" not in sys.path:
    sys.path.insert(0, "/opt/trn_rl_repo")

import numpy as np

import concourse.bacc as bacc
import concourse.mybir as mybir
from concourse.bass_utils import run_bass_kernel_spmd
from concourse.tile import TileContext

N_CORES = 8
PDIM = 128
N_RELS = 64
B = 1152  # edges per (tile, partition) cell

AF = mybir.ActivationFunctionType
OP = mybir.AluOpType
F16 = mybir.dt.float16
F32 = mybir.dt.float32

# --------------------------------------------------------------------------
# Custom fused DVE op (same registration machinery as production dve_ops).
# --------------------------------------------------------------------------
_OPS_CACHE: dict = {}


def _custom_ops():
    if _OPS_CACHE:
        return _OPS_CACHE
    from concourse import dve_ops
    from concourse.dve_ops import DveOp, OPS, _SUB_OPCODE_FOR_NAME
    from concourse.dve_spec import (
        C0,
        C1,
        One,
        Spec,
        Src0,
        Src1,
        _has_src1,
        lower,
        maxx,
    )
    from concourse.dve_uop import DveOpSpec

    existing = {op.name: op for op in OPS}

    def mk(key, name, body):
        if name in existing:
            _OPS_CACHE[key] = existing[name]
            return
        if name not in _SUB_OPCODE_FOR_NAME:
            row = max(_SUB_OPCODE_FOR_NAME.values()) + 1
            assert row < 0x20, "custom DVE opcode rows exhausted"
            _SUB_OPCODE_FOR_NAME[name] = row
        spec = Spec(body=body)
        shas = {}
        for ver in ("v3", "v4"):
            uops = lower(spec, ver=ver)
            s = DveOpSpec(
                name=name,
                opcode=_SUB_OPCODE_FOR_NAME[name],
                uops=uops,
                rd1_en=_has_src1(spec),
            )
            shas[ver] = s.sha(ver)
        op = DveOp(name, spec, subdim=False, uops_sha=shas)
        OPS.append(op)
        dve_ops.CUSTOM_DVE_SPECS[name] = spec
        _OPS_CACHE[key] = op

    # tau candidates: max((sm - mn - 1)*0.5, (sm - 1)/3);  in0=sm, in1=mn
    mk("tau_a", "ANT_TAU_A", maxx((Src0 - Src1 - One) * C0, (Src0 - One) * C1))
    # alpha0 = b*c + (1-b)*p with per-partition scalar APs s0=b, s1=(1-b)
    mk("aff2", "ANT_AFF2", Src0 * C0 + Src1 * C1)
    return _OPS_CACHE


# --------------------------------------------------------------------------
# Bass program
# --------------------------------------------------------------------------
_PROG_CACHE: dict = {}


def _build_program(z_eps: float, scale_factor: float, T: int):
    ops = _custom_ops()
    nc = bacc.Bacc(
        "TRN2",
        target_bir_lowering=False,
        num_devices=N_CORES,
        dynamic_dma_scratch_size=8192,
    )

    # Input stream per tile/partition: [ch0 ch1 ch2 | pp0 pp1 pp2] fp16
    xin_d = nc.dram_tensor("xin", [T * PDIM * 6 * B], F16, kind="ExternalInput")
    # Per (tile, partition) scalars: M00..M22, b0..b2, (1-b)0..(1-b)2, pad
    scl_d = nc.dram_tensor("scl", [T * PDIM * 16], F32, kind="ExternalInput")
    out_d = nc.dram_tensor("alpha", [T * PDIM * 3 * B], F16, kind="ExternalOutput")

    xin_v = xin_d[:].rearrange("(t p c) -> t p c", t=T, p=PDIM)
    scl_v = scl_d[:].rearrange("(t p c) -> t p c", t=T, p=PDIM)
    out_v = out_d[:].rearrange("(t p c) -> t p c", t=T, p=PDIM)

    V = nc.vector
    A = nc.scalar

    with TileContext(nc) as tc:
        with (
            nc.allow_low_precision(reason="fp16 pipeline; 2e-2 rel tolerance"),
            tc.tile_pool(name="io", bufs=2) as iop,
            tc.tile_pool(name="scr", bufs=2) as sp,
        ):
            for t in range(T):
                xin = iop.tile([PDIM, 6 * B], F16, tag="xin", name=f"xin{t}")
                scl = iop.tile([PDIM, 16], F32, tag="scl", name=f"scl{t}")
                ot = iop.tile([PDIM, 3 * B], F16, tag="ot", name=f"ot{t}")
                nc.sync.dma_start(xin[:], xin_v[t])
                nc.sync.dma_start(scl[:], scl_v[t])

                X = xin[:, 0 : 3 * B]  # child planes
                Pr = xin[:, 3 * B : 6 * B]  # parent (raw) planes

                def w6(tag, bufs=2):
                    return sp.tile(
                        [PDIM, 6 * B], F16, tag=tag, name=f"{tag}{t}", bufs=bufs
                    )[:]

                def w3(tag, bufs=1):
                    return sp.tile(
                        [PDIM, 3 * B], F16, tag=tag, name=f"{tag}{t}", bufs=bufs
                    )[:]

                def w2(tag, bufs=2):
                    return sp.tile(
                        [PDIM, 2 * B], F16, tag=tag, name=f"{tag}{t}", bufs=bufs
                    )[:]

                def pl(tag, bufs=1):
                    return sp.tile(
                        [PDIM, B], F16, tag=tag, name=f"{tag}{t}", bufs=bufs
                    )[:]

                def p3(x, i):
                    return x[:, i * B : (i + 1) * B]

                def sv(j):
                    return scl[:, j : j + 1]

                W = w6("W")  # [c_raw | p_raw] then later scratch
                SPX = w6("SPX")  # [c | p] sparsemax outputs
                YL = w6("YL", bufs=1)  # [y | y ln y]
                W9 = sp.tile([PDIM, 9 * B], F16, tag="W9", name=f"W9{t}", bufs=1)[:]

                CRw = W[:, 0 : 3 * B]
                # ---- c_raw_i = sum_j M_ij * ch_j  (per-partition M scalars)
                tmp = pl("mvt")
                for i in range(3):
                    V.tensor_scalar(p3(CRw, i), p3(X, 0), sv(3 * i), None, OP.mult)
                    V.tensor_scalar(tmp, p3(X, 1), sv(3 * i + 1), None, OP.mult)
                    V.tensor_add(p3(CRw, i), p3(CRw, i), tmp)
                    V.tensor_scalar(tmp, p3(X, 2), sv(3 * i + 2), None, OP.mult)
                    V.tensor_add(p3(CRw, i), p3(CRw, i), tmp)
                # copy p_raw next to c_raw (4x tensor_copy)
                V.tensor_copy(W[:, 3 * B : 6 * B], Pr)

                # ---- fused two-branch sparsemax: SPX = relu(W - tau)
                # strided views: component j of both branches = [128, 2, B]
                Wv = W.rearrange("p (u b) -> p u b", b=B)  # u = 6

                def comp(x6, j):
                    v = x6.rearrange("p (u b) -> p u b", b=B)
                    return v[:, j : j + 4 : 3, :]  # planes {j, j+3}

                mx = w2("mx")
                mn = w2("mn")
                sm = w2("sm")
                ta = w2("ta")
                mx2 = mx.rearrange("p (u b) -> p u b", b=B)
                mn2 = mn.rearrange("p (u b) -> p u b", b=B)
                sm2 = sm.rearrange("p (u b) -> p u b", b=B)
                ta2 = ta.rearrange("p (u b) -> p u b", b=B)
                x0, x1, x2 = comp(W, 0), comp(W, 1), comp(W, 2)
                V.tensor_max(mx2, x0, x1)
                V.tensor_tensor(mn2, x0, x1, OP.min)
                V.tensor_add(sm2, x0, x1)
                V.tensor_max(mx2, mx2, x2)
                V.tensor_tensor(mn2, mn2, x2, OP.min)
                V.tensor_add(sm2, sm2, x2)
                V._custom_dve(ops["tau_a"], out=ta, in0=sm, in1=mn, s0=0.5, s1=1.0 / 3.0)
                V.tensor_scalar(mx, mx, -1.0, None, OP.add)  # mx-1 (4x)
                V.tensor_max(ta, ta, mx)  # tau for both branches
                for j in range(3):
                    V.tensor_sub(comp(SPX, j), comp(W, j), ta2)
                V.tensor_scalar(SPX, SPX, 0.0, None, OP.max)  # relu (4x wide)

                Cc = SPX[:, 0 : 3 * B]
                Pp = SPX[:, 3 * B : 6 * B]

                # ---- cosine pieces: W9 = [p*c | p^2 | c^2], pq = sums
                V.tensor_mul(W9[:, 0 : 3 * B], Pp, Cc)
                A.square(W9[:, 3 * B : 6 * B], Pp)
                A.square(W9[:, 6 * B : 9 * B], Cc)
                pq = w3("pq")
                pq3 = pq.rearrange("p (u b) -> p u b", b=B)
                W9v = W9.rearrange("p (u b) -> p u b", b=B)
                V.tensor_add(pq3, W9v[:, 0:9:3, :], W9v[:, 1:9:3, :])
                V.tensor_add(pq3, pq3, W9v[:, 2:9:3, :])
                pc = pq[:, 0:B]
                pps = pq[:, B : 2 * B]
                ccs = pq[:, 2 * B : 3 * B]

                # ---- y = max(p + c, eps); l = y ln y
                Y = YL[:, 0 : 3 * B]
                LNp = YL[:, 3 * B : 6 * B]
                V.tensor_add(Y, Pp, Cc)
                V.tensor_scalar(Y, Y, float(z_eps), None, OP.max)
                A.activation(LNp, Y, AF.Ln)
                V.tensor_mul(LNp, Y, LNp)
                # zq = [zs | szl] via strided pair-sums over YL
                zq = w2("zq", bufs=1)
                zq2 = zq.rearrange("p (u b) -> p u b", b=B)
                YLv = YL.rearrange("p (u b) -> p u b", b=B)
                V.tensor_add(zq2, YLv[:, 0:6:3, :], YLv[:, 1:6:3, :])
                V.tensor_add(zq2, zq2, YLv[:, 2:6:3, :])
                zs = zq[:, 0:B]
                szl = zq[:, B : 2 * B]

                # ---- entropy = ln zs - szl/zs ; cos = 0.1 + pc/sqrt(pps*ccs)
                lzs = pl("lzs")
                izs = pl("izs")
                ent = pl("ent")
                nn = pl("nn")
                irt = pl("irt")
                ie = pl("ie")
                sc = pl("sc")
                V.tensor_mul(nn, pps, ccs)
                # ACT ops grouped by table: {Ln x3} then {Exp x2} then Ln/Exp
                A.activation(lzs, zs, AF.Ln)
                A.activation(irt, nn, AF.Ln)
                A.activation(izs, lzs, AF.Exp, scale=-1.0)  # 1/zs
                A.activation(irt, irt, AF.Exp, scale=-0.5)  # 1/sqrt(nn)
                V.tensor_mul(szl, szl, izs)
                V.tensor_sub(ent, lzs, szl)
                V.tensor_mul(pc, pc, irt)
                V.tensor_scalar(pc, pc, 0.1, None, OP.add)  # cos (4x)
                A.activation(ent, ent, AF.Ln)
                A.activation(ie, ent, AF.Exp, scale=-1.0)  # 1/ent
                V.tensor_mul(sc, pc, ie)  # cos/ent (x21 folded into final op)

                # ---- alpha = max(21 * (cos/ent) * (b*c + (1-b)*p), 0.001)
                # alpha0 = b*c + (1-b)*p as one custom op; b planes need a
                # single per-partition scalar each, so do it per component.
                for i in range(3):
                    V._custom_dve(
                        ops["aff2"],
                        out=p3(Y, i),
                        in0=p3(Cc, i),
                        in1=p3(Pp, i),
                        s0=sv(9 + i),
                        s1=sv(12 + i),
                    )
                for i in range(3):
                    V.tensor_mul(p3(Y, i), p3(Y, i), sc)
                V.tensor_scalar(ot[:], Y, float(scale_factor), 0.001, OP.mult, OP.max)

                nc.sync.dma_start(out_v[t], ot[:])

    nc.compile()
    return nc


def _get_program(z_eps: float, scale_factor: float, T: int):
    key = (round(z_eps, 9), round(scale_factor, 9), T)
    if key not in _PROG_CACHE:
        _PROG_CACHE[key] = _build_program(z_eps, scale_factor, T)
    return _PROG_CACHE[key]


# --------------------------------------------------------------------------
# Host-side pack/unpack
# --------------------------------------------------------------------------
def _pack_core(ch, pp, rels, t15, k, e, T):
    """Returns (xin_f16, scl_f32, order, gi, valid)."""
    sl = slice(k * e, (k + 1) * e)
    r = rels[sl]
    order = np.argsort(r, kind="stable")
    counts = np.bincount(r, minlength=N_RELS)
    nch = (counts + B - 1) // B
    cell_rel = np.repeat(np.arange(N_RELS, dtype=np.int64), nch)
    within = np.concatenate([np.arange(n, dtype=np.int64) * B for n in nch])
    ncells = cell_rel.shape[0]
    ncap = T * PDIM
    assert ncells <= ncap, (ncells, ncap)
    rel_starts = np.concatenate([[0], np.cumsum(counts)[:-1]])
    cell_start = rel_starts[cell_rel] + within
    cell_len = np.minimum(counts[cell_rel] - within, B)
    pad = ncap - ncells
    if pad:
        cell_rel = np.concatenate([cell_rel, np.zeros(pad, np.int64)])
        cell_start = np.concatenate([cell_start, np.zeros(pad, np.int64)])
        cell_len = np.concatenate([cell_len, np.zeros(pad, np.int64)])
    gi = cell_start[:, None] + np.arange(B, dtype=np.int64)[None, :]
    valid = np.arange(B, dtype=np.int64)[None, :] < cell_len[:, None]
    gi = np.where(valid, gi, 0)

    def pack(a):
        s = np.ascontiguousarray(a[sl], dtype=np.float32)[order]  # [e, 3]
        cells = s[gi]  # [ncap, B, 3]
        cells *= valid[..., None]
        return cells.transpose(0, 2, 1).reshape(T, PDIM, 3 * B).astype(np.float16)

    xin = np.concatenate([pack(ch), pack(pp)], axis=2).reshape(-1)
    scl = np.zeros((ncap, 16), dtype=np.float32)
    scl[:, :15] = t15[cell_rel]
    return xin, scl.reshape(-1), order, gi, valid


def _unpack_core(out_f16, order, gi, valid, e, T):
    cells = out_f16.reshape(T * PDIM, 3, B).transpose(0, 2, 1).astype(np.float32)
    res_sorted = np.empty((e, 3), dtype=np.float32)
    res_sorted[gi[valid]] = cells[valid]
    res = np.empty((e, 3), dtype=np.float32)
    res[order] = res_sorted
    return res


def _run(inputs: dict, trace: bool = False):
    ch = np.asarray(inputs["child_probs"], dtype=np.float32)
    pp = np.asarray(inputs["prnt_probs"], dtype=np.float32)
    M = np.asarray(inputs["M"], dtype=np.float32)
    beta = np.asarray(inputs["beta"], dtype=np.float32)
    rels = np.asarray(inputs["rels"]).astype(np.int64)
    z_eps = float(np.asarray(inputs["z_epsilon"]))
    sf = float(np.asarray(inputs["scale_factor"]))

    n = rels.shape[0]
    assert n % N_CORES == 0
    e = n // N_CORES

    t15 = np.concatenate(
        [M.reshape(N_RELS, 9), beta, 1.0 - beta], axis=1
    ).astype(np.float32)

    max_cells = 0
    for k in range(N_CORES):
        counts = np.bincount(rels[k * e : (k + 1) * e], minlength=N_RELS)
        max_cells = max(max_cells, int(((counts + B - 1) // B).sum()))
    T = max(1, -(-max_cells // PDIM))

    packs = [_pack_core(ch, pp, rels, t15, k, e, T) for k in range(N_CORES)]
    nc = _get_program(z_eps, sf, T)
    in_maps = [{"xin": p[0], "scl": p[1]} for p in packs]
    res = run_bass_kernel_spmd(nc, in_maps, core_ids=list(range(N_CORES)), trace=trace)
    outs = [
        _unpack_core(res.results[k]["alpha"], packs[k][2], packs[k][3], packs[k][4], e, T)
        for k in range(N_CORES)
    ]
    return np.concatenate(outs, axis=0), res


def kernel(**inputs) -> np.ndarray:
    out, _ = _run(inputs)
    return out


def kernel_traced(**inputs):
    """Returns (output, BassKernelResults-with-profile) for test harnesses."""
    return _run(inputs, trace=True)
